# revision 7
# baseline (speedup 1.0000x reference)
"""Trainium2 Bass kernel for y[b,o] = sum_k w[o,k] * x[b, idx[o,k]].

B=32, N_IN=1e6, N_OUT=5e5, K=3  (f32 end to end).

Sharding: 2 batch-groups of 16 rows x 4 output-quarters -> 8 cores.
Core c: batch rows [16G, 16G+16) (G=c//4), outputs [125000*Q, 125000*(Q+1))
(Q=c%4).

Per-core pipeline:
  Stage 1: x split into 64 windows of 16384 dofs; 8 windows in flight across
    the 8 gpsimd partition-groups (16 batch channels each). One ap_gather per
    round pulls every contribution's batch-column out of the SBUF-resident
    windows, bin-padded by (window, output-sub) so the layout is uniform; the
    result is one DMA per window into an HBM contribution buffer C.
  Stage 2: for each output-sub of 4096 outputs (8 subs in flight), its 64
    window-bins are loaded from C at static offsets, ap_gather re-orders them
    into (o, k) order, VectorE multiplies by w and reduces K=3, and the 16
    batch rows stream to y.

The program is compiled per input: PB (bin capacity) is computed from the
actual index histogram, so no overflow is possible.
"""
import numpy as np

B = 32
N_IN = 1_000_000
N_OUT = 500_000
K = 3

N_IN_PAD = 1 << 20       # 64 windows * 16384
WIN = 16384              # dofs per window
NW = 64                  # windows
NR = 8                   # stage-1 rounds (8 windows in flight)
NO_CORE = 125_000        # outputs per core
SUB = 4096               # outputs per sub
NS = 32                  # subs (32*4096 = 131072 >= 125000)
S2R = 4                  # stage-2 rounds (8 subs in flight)
NI2 = SUB * K            # stage-2 idxs per sub = 12288

_CACHE = {}


def _ceil_to(x, m):
    return (x + m - 1) // m * m


def _wrap16(a2):
    """[..., n] -> [..., 16, n//16]: partition j holds a[j::16]."""
    sh = a2.shape[:-1]
    n = a2.shape[-1]
    return np.ascontiguousarray(
        a2.reshape(*sh, n // 16, 16).swapaxes(-1, -2)
    )


def _prep_core(idx_c, w_c, pb=None):
    """Host-side binning for one core's 125000 outputs."""
    no = idx_c.shape[0]
    dof = idx_c.reshape(-1).astype(np.int64)          # [no*K], (o,k) order
    j_o = np.repeat(np.arange(no, dtype=np.int64), K)
    wid = dof >> 14
    loc = (dof & (WIN - 1)).astype(np.int64)
    sub = j_o >> 12

    binid = wid * NS + sub
    order = np.lexsort((np.arange(dof.size), binid))
    bin_sizes = np.bincount(binid, minlength=NW * NS)
    bin_starts = np.concatenate([[0], np.cumsum(bin_sizes)])
    rank = np.empty(dof.size, dtype=np.int64)
    rank[order] = np.arange(dof.size) - bin_starts[binid[order]]

    max_bin = int(bin_sizes.max())
    if pb is None:
        pb = int(_ceil_to(max_bin, 16))
    assert max_bin <= pb and NW * pb <= 32768  # int16 slot range

    # stage-1 idx lists: [NW, NS*pb] with bin (w,s) at columns [s*pb, s*pb+n)
    s1 = np.zeros((NW, NS * pb), dtype=np.int16)
    flatpos = wid * (NS * pb) + sub * pb + rank
    s1.reshape(-1)[flatpos] = loc.astype(np.int16)

    # stage-2 slots (natural (o,k) order): slot within csub = wid*pb + rank
    slots = (wid * pb + rank).astype(np.int16)

    s2i = np.zeros((S2R, 8, NI2), dtype=np.int16)
    wrep = np.zeros((S2R, 8, NI2), dtype=np.float32)
    w_flat = w_c.reshape(-1).astype(np.float32)
    for s in range(NS):
        r2, u = divmod(s, 8)
        j0 = s * NI2
        j1 = min((s + 1) * NI2, dof.size)
        if j1 > j0:
            s2i[r2, u, : j1 - j0] = slots[j0:j1]
            wrep[r2, u, : j1 - j0] = w_flat[j0:j1]
    return {"pb": pb, "max_bin": max_bin, "s1": s1, "s2i": s2i, "wrep": wrep}


def _build_nc(pb):
    import concourse.bacc as bacc
    import concourse.tile as tile
    import concourse.mybir as mybir
    
    ni1 = NS * pb            # stage-1 num_idxs per window
    cw = NW * pb             # csub columns = stage-2 num_elems

    nc = bacc.Bacc("TRN2", target_bir_lowering=False, debug=False, num_devices=8)
    xg_d = nc.dram_tensor("xg", [16, N_IN_PAD], mybir.dt.float32, kind="ExternalInput")
    s1i_d = nc.dram_tensor("s1i", [NR, 128, ni1 // 16], mybir.dt.int16, kind="ExternalInput")
    s2i_d = nc.dram_tensor("s2i", [S2R, 128, NI2 // 16], mybir.dt.int16, kind="ExternalInput")
    wr_d = nc.dram_tensor("wr", [S2R, 128, NI2], mybir.dt.float32, kind="ExternalInput")
    y_d = nc.dram_tensor("y", [16, NS * SUB], mybir.dt.float32, kind="ExternalOutput")
    c_d = nc.dram_tensor("cbuf", [NW, 16, ni1], mybir.dt.float32)

    with tile.TileContext(nc) as tc:
      with tc.tile_pool(name="p1", bufs=2) as p1:
        for r in range(NR):
            xwin = p1.tile([128, WIN], mybir.dt.float32)
            for u in range(8):
                wv = r * 8 + u
                nc.sync.dma_start(
                    xwin[16 * u : 16 * u + 16, :],
                    xg_d.ap()[:, wv * WIN : (wv + 1) * WIN],
                )
            s1idx = p1.tile([128, ni1 // 16], mybir.dt.int16)
            nc.sync.dma_start(s1idx[:], s1i_d.ap()[r])
            g1 = p1.tile([128, ni1], mybir.dt.float32)
            nc.gpsimd.ap_gather(
                out_ap=g1[:].rearrange("p (n d) -> p n d", d=1),
                in_ap=xwin[:].rearrange("p (n d) -> p n d", d=1),
                idxs_ap=s1idx[:],
                channels=128,
                num_elems=WIN,
                d=1,
                num_idxs=ni1,
            )
            for u in range(8):
                wv = r * 8 + u
                nc.scalar.dma_start(c_d.ap()[wv], g1[16 * u : 16 * u + 16, :])

      with tc.tile_pool(name="p2", bufs=1) as p2:
        for r2 in range(S2R):
            csub = p2.tile([128, cw], mybir.dt.float32)
            for u in range(8):
                s = r2 * 8 + u
                for wv in range(NW):
                    nc.sync.dma_start(
                        csub[16 * u : 16 * u + 16, wv * pb : (wv + 1) * pb],
                        c_d.ap()[wv][:, s * pb : (s + 1) * pb],
                    )
            s2idx = p2.tile([128, NI2 // 16], mybir.dt.int16)
            nc.sync.dma_start(s2idx[:], s2i_d.ap()[r2])
            wt = p2.tile([128, NI2], mybir.dt.float32)
            nc.sync.dma_start(wt[:], wr_d.ap()[r2])
            g2 = p2.tile([128, NI2], mybir.dt.float32)
            nc.gpsimd.ap_gather(
                out_ap=g2[:].rearrange("p (n d) -> p n d", d=1),
                in_ap=csub[:].rearrange("p (n d) -> p n d", d=1),
                idxs_ap=s2idx[:],
                channels=128,
                num_elems=cw,
                d=1,
                num_idxs=NI2,
            )
            nc.vector.tensor_tensor(
                out=g2[:], in0=g2[:], in1=wt[:], op=mybir.AluOpType.mult
            )
            yt = p2.tile([128, SUB], mybir.dt.float32)
            nc.vector.tensor_reduce(
                out=yt[:],
                in_=g2[:].rearrange("p (o k) -> p o k", k=K),
                axis=mybir.AxisListType.X,
                op=mybir.AluOpType.add,
            )
            for u in range(8):
                s = r2 * 8 + u
                nc.scalar.dma_start(
                    y_d.ap()[:, s * SUB : (s + 1) * SUB], yt[16 * u : 16 * u + 16, :]
                )
    nc.compile()
    return nc


def kernel(x, w, idx):
    from concourse.bass_utils import run_bass_kernel_spmd

    x = np.asarray(x, dtype=np.float32)
    w = np.asarray(w, dtype=np.float32)
    idx = np.asarray(idx)
    xpad = np.zeros((B, N_IN_PAD), dtype=np.float32)
    xpad[:, :N_IN] = x

    preps = [
        _prep_core(idx[c % 4 * NO_CORE : (c % 4 + 1) * NO_CORE],
                   w[c % 4 * NO_CORE : (c % 4 + 1) * NO_CORE])
        for c in range(8)
    ]
    pb = int(_ceil_to(max(p["max_bin"] for p in preps), 16))
    if any(p["pb"] != pb for p in preps):
        preps = [
            _prep_core(idx[c % 4 * NO_CORE : (c % 4 + 1) * NO_CORE],
                       w[c % 4 * NO_CORE : (c % 4 + 1) * NO_CORE], pb=pb)
            for c in range(8)
        ]

    if pb not in _CACHE:
        _CACHE[pb] = _build_nc(pb)
    nc = _CACHE[pb]

    ni1 = NS * pb
    in_maps = []
    for c in range(8):
        p = preps[c]
        g = c // 4
        s1i = np.zeros((NR, 128, ni1 // 16), dtype=np.int16)
        for wv in range(NW):
            r, u = divmod(wv, 8)
            s1i[r, 16 * u : 16 * u + 16, :] = _wrap16(p["s1"][wv])
        s2i = np.zeros((S2R, 128, NI2 // 16), dtype=np.int16)
        wrr = np.zeros((S2R, 128, NI2), dtype=np.float32)
        for r2 in range(S2R):
            for u in range(8):
                s2i[r2, 16 * u : 16 * u + 16, :] = _wrap16(p["s2i"][r2, u])
                wrr[r2, 16 * u : 16 * u + 16, :] = p["wrep"][r2, u][None, :]
        in_maps.append(
            {"xg": xpad[16 * g : 16 * g + 16], "s1i": s1i, "s2i": s2i, "wr": wrr}
        )

    res = run_bass_kernel_spmd(nc, in_maps, core_ids=list(range(8)))
    kernel._last_exec_ns = res.exec_time_ns
    y = np.zeros((B, N_OUT), dtype=np.float32)
    for c in range(8):
        g, q = c // 4, c % 4
        y[16 * g : 16 * g + 16, q * NO_CORE : (q + 1) * NO_CORE] = \
            res.results[c]["y"][:, :NO_CORE]
    return y
